# revision 22
# baseline (speedup 1.0000x reference)
"""Trainium2 Bass kernel for MultiHeadAttentionWithRope.

Problem: B=2, T=2048, C=2048, H=16 heads, D=128 head_dim, fp32 I/O.
  qkv = x @ W_qkv; q,k -> RoPE (adjacent-pair, torchtune) -> causal SDPA
  -> out = o @ W_out

Sharding (8 cores): 2 batches x 4 head-groups (4 heads each).
Each core computes a partial output out_partial[b] = o_heads @ W_out_rows
(bf16); the host sums the 4 partials per batch in f32.

Per-core layout trick: everything is computed in "transposed" space.
  - host pre-transposes x[b] -> xT [C, T] (bf16)
  - qT,kT = (W_q|k)^T x in [D, T] layout directly (lhsT = W slice as stored)
  - RoPE dims are de-interleaved by permuting W_qkv q/k columns on the host
    (even dims first). S = q.k is invariant under a shared permutation.
  - scores are computed transposed: S.T[k,q] = matmul(lhsT=kT, rhs=qT),
    so softmax probs P~[k,q] feed the PV matmul with no on-device transpose.
  - no max-subtraction in softmax (logits are bounded: |S|<~6 at this scale)
  - row-sums l: probs chunks are pairwise tree-summed in bf16 on DVE/Pool,
    then ONE ones-lhsT matmul per (qt,h) group reduces over partitions.
  - 1/l via DVE reciprocal of the lp PSUM tile (all 128 rows of lp are
    identical because the ones lhsT is all-ones), so normalization is
    lp MM -> DVE reciprocal -> DVE multiply. No ACT ln/exp chain, no
    PE broadcast matmul.
All matmuls in bf16 (fp32 accumulate in PSUM).

Schedule: single fused pipeline. For each 512-token block tb:
  QKV+RoPE(tb) -> V(tb, kc-outer) -> attention(qt=tb) over k-chunks
  0..4(tb+1)-1, with out-proj units for qt-1 dripped in as PE filler.
The PE stream never has a phase boundary; ACT's exp stream (the second
busiest engine) overlaps the QKV matmuls of the next block. Diagonal
prob tiles with masked prefixes are persistent and pre-zeroed once, so
no per-chunk memsets exist in the steady state.

Startup: per-chunk x/w tiles (fine-grained DMA deps) and QKV matmuls
emitted in groups of 4 heads kc-outer, so the PE consumes chunks at DMA
arrival pace instead of waiting for whole tensors.
"""

import sys

sys.path.insert(0, "/opt/trn_rl_repo")

import numpy as np
import ml_dtypes

import concourse.bass as bass
import concourse.tile as tile
from concourse import mybir
from concourse.bass import ts
from concourse.bass_utils import run_bass_kernel_spmd

# Provide antenv.axon_hooks (absent in this container) so trace=True can use
# the axon NTFF profiling path.
def _ensure_axon_hooks():
    import types

    try:
        from antenv import axon_hooks  # noqa: F401
        return
    except ImportError:
        pass
    import antenv

    mod = types.ModuleType("antenv.axon_hooks")
    mod._hook = None

    def set_axon_ntff_profile_hook(h):
        mod._hook = h

    def get_axon_ntff_profile_hook():
        return mod._hook

    mod.set_axon_ntff_profile_hook = set_axon_ntff_profile_hook
    mod.get_axon_ntff_profile_hook = get_axon_ntff_profile_hook
    sys.modules["antenv.axon_hooks"] = mod
    antenv.axon_hooks = mod
    try:
        from trn_agent_boot.trn_boot import _ntff_profile_via_ctypes

        hook = _ntff_profile_via_ctypes("/opt/axon/libaxon_pjrt.so")
        if hook is not None:
            mod._hook = hook
    except Exception:
        pass


_ensure_axon_hooks()

# ---------------------------------------------------------------------------
# This walrus build supports only ONE sync-wait command per instruction.
# TileContext's sem assignment can attach several waits to one instruction
# (and its exit drain aggregates many). Post-pass: hoist excess waits onto
# same-engine NoOps inserted immediately before the instruction -- the
# engine blocks on each wait in order, so semantics are identical.
MAX_WAITS_PER_INST = 1


_ALL_ENGINES = [
    mybir.EngineType.PE,
    mybir.EngineType.Activation,
    mybir.EngineType.DVE,
    mybir.EngineType.Pool,
    mybir.EngineType.SP,
]


def _split_sync_waits(nc):
    for f in nc.m.functions:
        for blk in f.blocks:
            new_insts = []
            for ins in blk.instructions:
                si = getattr(ins, "sync_info", None)
                lim = 1 if isinstance(ins, mybir.InstDrain) else MAX_WAITS_PER_INST
                if si is not None and si.on_wait and len(si.on_wait) > lim:
                    waits = list(si.on_wait)
                    keep = waits[:lim]
                    extra = waits[lim:]
                    # A drain with a big wait-set is the kernel-tail barrier:
                    # spread its waits across all engines so they resolve in
                    # parallel (the all-engine barrier right after joins them).
                    spread = (
                        isinstance(ins, mybir.InstDrain) and len(extra) > 4
                    )
                    for i, w in enumerate(extra):
                        eng = (
                            _ALL_ENGINES[i % len(_ALL_ENGINES)]
                            if spread
                            else ins.engine
                        )
                        nop = mybir.InstNoOp(
                            name=nc.get_next_instruction_name(),
                            sync_info=mybir.SyncInfo(on_wait=[w], on_update=[]),
                            bass_nofuse=True,
                            engine=eng,
                        )
                        new_insts.append(nop)
                    si.on_wait = keep
                new_insts.append(ins)
            if len(new_insts) != len(blk.instructions):
                blk.instructions = new_insts


# ---------------------------------------------------------------------------

B, T, C, H = 2, 2048, 2048, 16
D = C // H  # 128
ROPE_BASE = 10000.0
HG = 4  # head groups
HL = H // HG  # heads per core = 4
CL = HL * D  # local width = 512
P = 128
TB = 512  # token block
NTB = T // TB  # 4
KCH = T // P  # 16 k-chunks
QT = T // TB  # 4 q-tiles
SCALE = 1.0 / float(np.sqrt(D))

BF16 = mybir.dt.bfloat16
F32 = mybir.dt.float32
bf16_np = ml_dtypes.bfloat16


def _build_nc():
    nc = bass.Bass(trn_type="TRN2")
    xT = nc.declare_dram_parameter("xT", [C, T], BF16, isOutput=False)
    wqkv = nc.declare_dram_parameter("wqkv", [C, 3 * CL], BF16, isOutput=False)
    wout = nc.declare_dram_parameter("wout", [CL, C], BF16, isOutput=False)
    tabs = nc.declare_dram_parameter("tabs", [2, P, T], BF16, isOutput=False)
    masks = nc.declare_dram_parameter("masks", [2, P, P], BF16, isOutput=False)
    out = nc.declare_dram_parameter("out", [T, C], BF16, isOutput=True)

    xT_r = xT[:].rearrange("(ko p) t -> p ko t", p=P)  # [128,16,T]
    wqkv_r = wqkv[:].rearrange("(ko p) m -> p ko m", p=P)  # [128,16,1536]
    wout_r = wout[:].rearrange("(h p) n -> p h n", p=P)  # [128,4,2048]
    out_r = out[:].rearrange("(tc p) n -> tc p n", p=P)  # [16,128,2048]

    with tile.TileContext(nc) as tc:
        consts = tc.alloc_tile_pool(name="consts", bufs=1)
        xcpool = tc.alloc_tile_pool(name="xcpool", bufs=20)
        prepool = tc.alloc_tile_pool(name="prepool", bufs=4)
        swppool = tc.alloc_tile_pool(name="swppool", bufs=4)
        ropepool = tc.alloc_tile_pool(name="ropepool", bufs=3)
        persist = tc.alloc_tile_pool(name="persist", bufs=1)
        ptpool = tc.alloc_tile_pool(name="ptpool", bufs=8)
        accpool = tc.alloc_tile_pool(name="accpool", bufs=7)
        rtpool = tc.alloc_tile_pool(name="rtpool", bufs=1)
        outpool = tc.alloc_tile_pool(name="outpool", bufs=4)
        mm_psum = tc.alloc_tile_pool(name="mm_psum", bufs=4, space="PSUM")
        acc_psum = tc.alloc_tile_pool(name="acc_psum", bufs=4, space="PSUM")

        # ---- HAM warmup ----
        # The PE's HAM clock gate starts cold (1.2 GHz, ~3.4us of sustained
        # activity to warm). Dummy matmuls on scratch data cover the first
        # DMA-arrival window so real matmuls run warm. warm_c/warm_acc are
        # also used for tiny keep-alive matmuls threaded through block 0's
        # DMA-paced stretch: they stop any 3.4us-idle window from dropping
        # the clock back to 1.2 GHz while the PE waits on input arrival.
        warm_c = consts.tile([P, TB], BF16)
        nc.gpsimd.memset(warm_c[:], 1.0)
        warm_acc = acc_psum.tile([P, TB], F32, tag="acc", name="warm")
        for _ in range(8):
            nc.tensor.matmul(
                warm_acc[:], lhsT=warm_c[:, :P], rhs=warm_c[:], start=True, stop=True
            )

        def keep_warm():
            nc.tensor.matmul(
                warm_acc[:, :P],
                lhsT=warm_c[:, :P],
                rhs=warm_c[:, :P],
                start=True,
                stop=True,
            )

        # ---- input DMAs, ordered by first use ----
        # Per-chunk DMAs round-robin across the hardware DMA engines and
        # transfer in parallel; one 128KB DMA instruction per k-chunk.
        # Queue plan: Sync streams x chunks (the pace-setter for block 0),
        # DVE/Pool split the q/k weights (both needed in block 0), the
        # first rope-table chunk goes out first on Pool (needed by the rope
        # of block 0 at ~18us), then w_v / remaining tables / masks.
        # NOTHING is issued on the ACT queue: DMA-issue instructions there
        # would head-of-line block the PSUM-freeing `pre` copies that gate
        # the next QKV psum chain group.
        cos_sb = consts.tile([P, T], BF16)
        sin_sb = consts.tile([P, T], BF16)
        xcv = [[None] * KCH for _ in range(NTB)]
        w_q = []
        w_k = []
        w_v = []
        # block 0 is DMA-arrival paced: spread its critical tensors (x0, w_q)
        # round-robin over all three DMA-capable queues in consumption order,
        # so each queue carries ~1/3 of the early-needed bytes
        Q3 = (nc.sync, nc.scalar, nc.gpsimd)
        for kc in range(KCH):
            w_q.append(consts.tile([P, CL], BF16, name=f"w_q{kc}"))
            w_k.append(consts.tile([P, CL], BF16, name=f"w_k{kc}"))
            w_v.append(consts.tile([P, CL], BF16, name=f"w_v{kc}"))
            xcv[0][kc] = xcpool.tile([P, TB], BF16, tag="xc", name=f"xc0_{kc}")
            Q3[kc % 3].dma_start(out=xcv[0][kc][:], in_=xT_r[:, kc, ts(0, TB)])
            Q3[(kc + 1) % 3].dma_start(out=w_q[kc][:], in_=wqkv_r[:, kc, 0:CL])
            if kc == 7:
                nc.gpsimd.dma_start(out=cos_sb[:, 0:TB], in_=tabs[0, :, 0:TB])
                nc.sync.dma_start(out=sin_sb[:, 0:TB], in_=tabs[1, :, 0:TB])
        ident_sb = consts.tile([P, P], BF16)
        nc.scalar.dma_start(out=ident_sb[:], in_=masks[0])
        tri_sb = consts.tile([P, P], BF16)
        nc.scalar.dma_start(out=tri_sb[:], in_=masks[1])
        for kc in range(KCH):
            Q3[(kc + 2) % 3].dma_start(out=w_k[kc][:], in_=wqkv_r[:, kc, CL : 2 * CL])
        for kc in range(KCH):
            Q3[kc % 3].dma_start(out=w_v[kc][:], in_=wqkv_r[:, kc, 2 * CL : 3 * CL])
        for tbc in range(1, NTB):
            nc.scalar.dma_start(out=cos_sb[:, ts(tbc, TB)], in_=tabs[0, :, ts(tbc, TB)])
            nc.scalar.dma_start(out=sin_sb[:, ts(tbc, TB)], in_=tabs[1, :, ts(tbc, TB)])
        ones_sb = consts.tile([P, P], BF16)
        nc.vector.memset(ones_sb[:], 1.0)
        wo_sb = consts.tile([P, HL, C], BF16)  # 2MB, loaded at block 1

        # Persistent diagonal prob tiles: the masked prefix [0:qs) is zeroed
        # exactly once here and never written again (exp only writes the
        # valid suffix), so the per-chunk memsets of the old schedule vanish.
        # Two sets alternate by global head parity.
        diag_pt = []
        for s in range(2):
            row = []
            for off in (1, 2, 3):
                t_ = consts.tile([P, TB], BF16, name=f"diag{s}_{off}")
                nc.gpsimd.memset(t_[:], 0.0)
                row.append(t_)
            diag_pt.append(row)

        # ---- persistent activations ----
        qT_sb = persist.tile([P, HL, T], BF16)  # 2MB
        kT_sb = persist.tile([P, HL, T], BF16)  # 2MB
        v_sb = persist.tile([P, KCH, CL], BF16)  # 2MB
        oT_sb = persist.tile([P, HL, T], BF16)  # 2MB

        # ---------------- out-proj drip queue ----------------
        # Out-proj units (one PSUM accumulation of 4 head matmuls + evict +
        # store DMA) are pure PE filler: their inputs are ready long before
        # emission, so they absorb any stall in the attention dependency
        # chain (the PE queue is strict in-order; a blocked matmul blocks
        # everything behind it -- filler must therefore never block).
        # A few units are held in reserve (tail=False skips them) to fill
        # the very last head's lp/normalize flush at the end of the kernel.
        oq = []  # (tcc, ncc)
        ev_ctr = [0]
        RESERVE = 4

        def emit_outproj_unit(tail=False):
            if len(oq) <= (0 if tail else RESERVE):
                return
            tcc, ncc = oq.pop(0)
            outp = mm_psum.tile([P, TB], F32, tag="mm")
            for h in range(HL):
                nc.tensor.matmul(
                    outp[:],
                    lhsT=oT_sb[:, h, ts(tcc, P)],
                    rhs=wo_sb[:, h, ts(ncc, TB)],
                    start=(h == 0),
                    stop=(h == HL - 1),
                )
            ot = outpool.tile([P, TB], BF16)
            ev_ctr[0] += 1
            # half-width eviction on ACT and DVE in parallel: the psum ring
            # slot frees in ~half the latency, so the next unit's first
            # matmul doesn't stall on a single busy engine's queue
            nc.scalar.copy(out=ot[:, : TB // 2], in_=outp[:, : TB // 2])
            nc.vector.tensor_copy(out=ot[:, TB // 2 :], in_=outp[:, TB // 2 :])
            # mid-kernel out stores ride the Sync queue only (the Pool queue
            # carries attention-critical tree adds that a DMA-issue would
            # head-of-line delay); the tail flush spreads across three queues
            # so the end-of-kernel drain isn't waiting on one queue's backlog
            dq = (nc.sync, nc.gpsimd, nc.scalar)[ev_ctr[0] % 3] if tail else nc.sync
            dq.dma_start(out=out_r[tcc, :, ts(ncc, TB)], in_=ot[:])

        def drip(n, tail=False):
            for _ in range(n):
                emit_outproj_unit(tail=tail)

        # Deferred per-head tails: every lp rowsum matmul and the
        # reciprocal+normalize of head h are emitted one FULL head (or one
        # QKV block) later, by which time the DVE tree adds they consume are
        # long done -- the in-order PE queue never waits on them.
        fin_steps = []  # list of closures

        def pop_fin(k=1):
            for _ in range(min(k, len(fin_steps))):
                fin_steps.pop(0)()

        def drain_fin():
            pop_fin(len(fin_steps))

        # ---------------- QKV + RoPE + V for one token block ----------------
        def emit_block(tb):
            # prefetch next block's x chunks; slots free progressively as the
            # kc-outer V group of this block consumes this block's chunks
            if tb + 1 < NTB:
                for kc in range(KCH):
                    xcv[tb + 1][kc] = xcpool.tile(
                        [P, TB], BF16, tag="xc", name=f"xc{tb + 1}_{kc}"
                    )
                    nc.sync.dma_start(
                        out=xcv[tb + 1][kc][:], in_=xT_r[:, kc, ts(tb + 1, TB)]
                    )

            # qT / kT in [D, token] layout + RoPE. Emitted kc-outer in groups
            # of 4 heads: 4 psum chains advance one chunk per DMA arrival, so
            # the PE streams at DMA pace instead of stalling on chunk 15.
            for mg in range(2):
                qps = [
                    mm_psum.tile([P, TB], F32, tag="mm", name=f"qp{tb}_{mg}_{i}")
                    for i in range(4)
                ]
                for kc in range(KCH):
                    for ml in range(4):
                        w_t = w_q[kc] if mg == 0 else w_k[kc]
                        nc.tensor.matmul(
                            qps[ml][:],
                            lhsT=w_t[:, ts(ml, P)],
                            rhs=xcv[tb][kc][:],
                            start=(kc == 0),
                            stop=(kc == KCH - 1),
                        )
                    if tb == 0:
                        keep_warm()
                if mg == 0 and tb > 0:
                    # previous q-tile's last head tail: leftover PVs, lp
                    # matmuls, reciprocal and normalize; its DVE/Pool tree
                    # adds finished during the mg0 chains above. Queue that
                    # q-tile's out-proj units, but drip the first one only
                    # after the mg1 chains (the normalize needs DVE time).
                    drain_fin()
                    oq.extend(
                        ((tb - 1) * (TB // P) + u // (C // TB), u % (C // TB))
                        for u in range((TB // P) * (C // TB))
                    )
                if mg == 1:
                    drip(2)
                for ml in range(4):
                    pre = prepool.tile([P, TB], BF16, tag="pre")
                    nc.vector.tensor_copy(out=pre[:], in_=qps[ml][:])
                    # swap partition halves via SBUF->SBUF DMA (DVE cannot
                    # cross partitions), issued on the Pool queue.
                    h64 = D // 2
                    swp = swppool.tile([P, TB], BF16, tag="swp")
                    nc.gpsimd.dma_start(out=swp[0:h64], in_=pre[h64 : 2 * h64])
                    nc.gpsimd.dma_start(out=swp[h64 : 2 * h64], in_=pre[0:h64])
                    ta = ropepool.tile([P, TB], BF16, tag="ta")
                    tb_ = ropepool.tile([P, TB], BF16, tag="tb")
                    # rope = pre * cosF + swap(pre) * sinS  (sinS = [-sin; +sin])
                    nc.vector.tensor_mul(ta[:], pre[:], cos_sb[:, ts(tb, TB)])
                    nc.gpsimd.tensor_mul(tb_[:], swp[:], sin_sb[:, ts(tb, TB)])
                    dest = (
                        qT_sb[:, ml, ts(tb, TB)]
                        if mg == 0
                        else kT_sb[:, ml, ts(tb, TB)]
                    )
                    nc.vector.tensor_add(dest[:], ta[:], tb_[:])
                if mg == 1:
                    drip(1)

            # V in natural [token, D] layout, kc-outer so x chunks release
            # progressively (unblocking the next block's prefetch DMAs)
            vps = [
                mm_psum.tile([P, TB], F32, tag="mm", name=f"vp{tb}_{i}")
                for i in range(4)
            ]
            for kc in range(KCH):
                for tsc in range(TB // P):
                    nc.tensor.matmul(
                        vps[tsc][:],
                        lhsT=xcv[tb][kc][:, ts(tsc, P)],
                        rhs=w_v[kc][:],
                        start=(kc == 0),
                        stop=(kc == KCH - 1),
                    )
            for tsc in range(TB // P):
                # split eviction engines: a DVE-only burst here would delay
                # the first score chunks of the next q-tile (their psum ring
                # slots wait on these reads)
                if tsc % 2 == 0:
                    nc.vector.tensor_copy(
                        out=v_sb[:, tb * (TB // P) + tsc, :], in_=vps[tsc][:]
                    )
                else:
                    nc.scalar.copy(
                        out=v_sb[:, tb * (TB // P) + tsc, :], in_=vps[tsc][:]
                    )
            if tb == 0:
                # wout load: issued on the ACT queue while it is still free of
                # exp work (before att(0)), data lands well before the first
                # out-proj drip in block 1
                for hh in range(HL):
                    nc.scalar.dma_start(
                        out=wo_sb[:, hh, ts(0, C // 2)],
                        in_=wout_r[:, hh, ts(0, C // 2)],
                    )
                    nc.scalar.dma_start(
                        out=wo_sb[:, hh, ts(1, C // 2)],
                        in_=wout_r[:, hh, ts(1, C // 2)],
                    )
            drip(2)

        # ---------------- attention for one q-tile ----------------
        head_ctr = [0]

        def emit_attention(qt, h):
            dset = diag_pt[head_ctr[0] % 2]
            head_ctr[0] += 1
            op = acc_psum.tile([P, TB], F32, tag="acc", name="op")
            lp = acc_psum.tile([P, TB], F32, tag="acc", name="lp")
            nkc = (qt + 1) * (TB // P)
            nl2 = nkc // 4
            s2s = []  # 4-chunk prob sums, consumed by deferred lp matmuls
            pair = None  # L1-pending prob chunk
            l1 = None  # L2-pending L1 sum
            pv_wait = []  # (pt, qs, kc): PV matmuls lagged two chunks
            for kc in range(nkc):
                # columns q < 128*off are entirely masked for this k-chunk;
                # restrict all work to the valid suffix [qs:TB)
                off = kc - qt * (TB // P)
                qs = max(off, 0) * P
                W = TB - qs
                sp = mm_psum.tile([P, TB], F32, tag="mm")
                nc.tensor.matmul(
                    sp[:, :W],
                    lhsT=kT_sb[:, h, ts(kc, P)],
                    rhs=qT_sb[:, h, qt * TB + qs : (qt + 1) * TB],
                    start=True,
                    stop=(off < 0),
                )
                if off >= 0:
                    # causal mask: accumulate a -1e9 upper-triangle bias into
                    # the 128 diagonal columns (one cheap PE matmul instead of
                    # a DVE multiply on the whole chunk)
                    nc.tensor.matmul(
                        sp[:, :P],
                        lhsT=ident_sb[:],
                        rhs=tri_sb[:],
                        start=False,
                        stop=True,
                    )
                if kc >= 1:
                    # one deferred step (leftover PV / lp matmul / normalize)
                    # of the PREVIOUS head: its operands are ~a head old
                    pop_fin(1)
                if off >= 1:
                    pt = dset[off - 1]  # persistent, prefix pre-zeroed
                else:
                    pt = ptpool.tile([P, TB], BF16, tag="pt")
                nc.scalar.activation(
                    out=pt[:, qs:],
                    in_=sp[:, :W],
                    func=mybir.ActivationFunctionType.Exp,
                    scale=SCALE,
                )
                # PV matmuls lag two chunks behind their exp so the ACT queue
                # (the second-busiest engine) has slack before the PE needs
                # its output
                pv_wait.append((pt, qs, kc))
                if len(pv_wait) > 2:
                    ppt, pqs, pkc = pv_wait.pop(0)
                    nc.tensor.matmul(
                        op[:, pqs:],
                        lhsT=v_sb[:, pkc, ts(h, P)],
                        rhs=ppt[:, pqs:],
                        start=(pkc == 0),
                        stop=False,
                    )
                if kc % 3 == 2:
                    drip(1)
                # pairwise 4-chunk prob sums in bf16 (magnitude-balanced,
                # ~0.1% on l); s1 adds ride the otherwise-idle Pool engine,
                # s2 on DVE; the consuming lp matmuls are deferred a full head
                if pair is None:
                    pair = pt
                else:
                    s1 = accpool.tile([P, TB], BF16, name="l1sum")
                    # DVE, not Pool: GpSimd 2-input ops are 2.7x slower AND
                    # contend with DVE for SBUF ports (observed DVE adds
                    # slowing 423->1367ns when overlapped with a Pool add)
                    nc.vector.tensor_add(s1[:], pair[:], pt[:])
                    pair = None
                    if l1 is None:
                        l1 = s1
                    else:
                        s2 = accpool.tile([P, TB], BF16, name="l2sum")
                        nc.vector.tensor_add(s2[:], l1[:], s1[:])
                        l1 = None
                        s2s.append(s2)
            # the final two PV matmuls become deferred steps as well: their
            # exps are still in the ACT queue at head end, so emitting them
            # here would head-of-line stall the PE
            def mk_pv(ppt, pqs, pkc):
                def step():
                    nc.tensor.matmul(
                        op[:, pqs:],
                        lhsT=v_sb[:, pkc, ts(h, P)],
                        rhs=ppt[:, pqs:],
                        start=(pkc == 0),
                        stop=(pkc == nkc - 1),
                    )
                return step

            fin_steps.extend(mk_pv(*pv) for pv in pv_wait)

            def mk_lp(g):
                def step():
                    nc.tensor.matmul(
                        lp[:],
                        lhsT=ones_sb[:],
                        rhs=s2s[g][:],
                        start=(g == 0),
                        stop=(g == nl2 - 1),
                    )
                return step

            def norm():
                # 1/l as exp(-ln(l)) on the ACT engine: two ~0.7us full-tile
                # ops (ACT time is free-dim-limited, so full-tile costs the
                # same as a single row). The IEEE-exact DVE reciprocal takes
                # 3.4us and head-of-line blocks the DVE queue; ACT's
                # Reciprocal/Ln_prime are unavailable (accuracy guard /
                # missing from the act tables).
                lt = rtpool.tile([P, TB], F32, tag="lt")
                nc.scalar.activation(
                    out=lt[:], in_=lp[:], func=mybir.ActivationFunctionType.Ln
                )
                rt = rtpool.tile([P, TB], F32, tag="rt")
                nc.scalar.activation(
                    out=rt[:],
                    in_=lt[:],
                    func=mybir.ActivationFunctionType.Exp,
                    scale=-1.0,
                )
                nc.vector.tensor_mul(oT_sb[:, h, ts(qt, TB)], op[:], rt[:])

            fin_steps.extend(mk_lp(g) for g in range(nl2))
            fin_steps.append(norm)

        # ---------------- fused pipeline ----------------
        for tb in range(NTB):
            emit_block(tb)
            for h in range(HL):
                emit_attention(tb, h)
                drip(1)
        # tail: reserved out-proj units bridge the last head's deferred tail
        drip(2, tail=True)
        pop_fin(2)  # leftover PVs (their exps finish during the drips)
        drip(1, tail=True)
        pop_fin(len(fin_steps) - 1)  # lp matmuls
        drip(1, tail=True)
        pop_fin(1)  # reciprocal + normalize of the last head
        oq.extend(
            ((NTB - 1) * (TB // P) + u // (C // TB), u % (C // TB))
            for u in range((TB // P) * (C // TB))
        )
        while oq:
            drip(1, tail=True)

        for pool in (
            acc_psum,
            mm_psum,
            outpool,
            rtpool,
            accpool,
            ptpool,
            persist,
            ropepool,
            swppool,
            prepool,
            xcpool,
            consts,
        ):
            pool.release()

    _split_sync_waits(nc)
    return nc


def _host_inputs(x, W_qkv, W_out):
    """Build per-core input maps. Core j: batch j//HG, head-group j%HG."""
    perm = np.concatenate([np.arange(0, D, 2), np.arange(1, D, 2)])  # deinterleave

    # rope tables in de-interleaved layout: rows [0:64]=even-dim freq, dup below
    inv = 1.0 / (ROPE_BASE ** (np.arange(0, D, 2, dtype=np.float32) / D))  # [64]
    ang = np.arange(T, dtype=np.float32)[None, :] * inv[:, None]  # [64, T]
    cosF = np.concatenate([np.cos(ang), np.cos(ang)], axis=0)  # [128, T]
    sinS = np.concatenate([-np.sin(ang), np.sin(ang)], axis=0)  # sign folded
    tabs = np.stack([cosF, sinS]).astype(bf16_np)  # [2,128,T]

    kk = np.arange(P)[:, None]
    qq = np.arange(P)[None, :]
    ident = np.eye(P, dtype=np.float32)
    tri = np.where(qq >= kk, 0.0, -1e9).astype(np.float32)
    mask = np.stack([ident, tri]).astype(bf16_np)  # [2,128,128]

    in_maps = []
    for j in range(8):
        b, hg = j // HG, j % HG
        xTb = np.ascontiguousarray(x[b].T).astype(bf16_np)  # [C, T]
        cols = []
        for part in range(2):  # q, k with permuted D
            for h in range(HL):
                base = part * C + (hg * HL + h) * D
                cols.append(W_qkv[:, base + perm])
        for h in range(HL):  # v natural
            base = 2 * C + (hg * HL + h) * D
            cols.append(W_qkv[:, base : base + D])
        wq = np.concatenate(cols, axis=1).astype(bf16_np)  # [C, 3*CL]
        wo = W_out[hg * CL : (hg + 1) * CL, :].astype(bf16_np)  # [CL, C]
        in_maps.append({"xT": xTb, "wqkv": wq, "wout": wo, "tabs": tabs, "masks": mask})
    return in_maps


def kernel(x, W_qkv, W_out, _trace=False, _tmpdir=None):
    x = np.asarray(x, dtype=np.float32)
    W_qkv = np.asarray(W_qkv, dtype=np.float32)
    W_out = np.asarray(W_out, dtype=np.float32)

    nc = _build_nc()
    in_maps = _host_inputs(x, W_qkv, W_out)
    res = run_bass_kernel_spmd(
        nc, in_maps, core_ids=list(range(8)), trace=_trace, tmpdir=_tmpdir
    )

    out = np.zeros((B, T, C), dtype=np.float32)
    for j in range(8):
        out[j // HG] += np.asarray(res.results[j]["out"], dtype=np.float32)
    if _trace:
        return out, res
    return out
